# revision 21
# baseline (speedup 1.0000x reference)
"""Multi-head self-attention (B=4, N=2048, F=1024, 16 heads) on 8 TRN2 NeuronCores.

Sharding (Megatron-style, per the hint): data-parallel over the 4 batches x
tensor-parallel split of the 16 heads into 2 groups of 8. Core c handles
batch c//2 with head group c%2 (512 of the 1024 qkv features, column-split
Wq/Wk/Wv, row-split Wo). Each core emits a partial o-projection [2048, 1024];
the host unshard sums the pair of partials per batch (the Megatron
all-reduce) and stacks batches.

Device kernel layout choices (all matmuls bf16 with f32 PSUM accumulation):
  xT   [1024f, 2048i]  (x transposed on host)  - rhs for q/k, lhsT for v
  qT/kT [512o, 2048i]  (o = head-major features, on partitions)
  vAug [2048j, 8h, 65] (per head: V columns 0..63 plus a ones column at 64
                        so the attention-value matmul also yields the softmax
                        denominator Z as output row 64)
  scores S^T [j, i] via lhsT=kT-chunk, rhs=qT; exp on ScalarE (scale=1/32,
  no max subtraction needed: |S/32| <~ 1.5); attnU^T accumulated over j in
  PSUM, copied to SBUF promptly to release the PSUM bank; softmax
  normalization (1/Z broadcast) runs off the critical path via small DMA
  round-trips through DRAM.

QKV is interleaved with attention per 128-row chunk of q/k so the ScalarE
exp stream (the steady-state bottleneck) starts early and the remaining
projections hide under it.
"""

import sys
import types

sys.path.insert(0, "/opt/trn_rl_repo")

import numpy as np

# Best-effort: register the axon NTFF profile hook so trace=True works
# (used by test harnesses via BASS_TRACE); harmless when tracing is off.
try:
    import antenv

    if "antenv.axon_hooks" not in sys.modules:
        from trn_agent_boot.trn_boot import _ntff_profile_via_ctypes

        _hooks_mod = types.ModuleType("antenv.axon_hooks")
        _hook = _ntff_profile_via_ctypes("/opt/axon/libaxon_pjrt.so")
        _hooks_mod.get_axon_ntff_profile_hook = lambda: _hook
        _hooks_mod.set_axon_ntff_profile_hook = lambda h: None
        sys.modules["antenv.axon_hooks"] = _hooks_mod
        antenv.axon_hooks = _hooks_mod
except Exception:
    pass

import concourse.bacc as bacc
import concourse.tile as tile
from concourse import mybir
from concourse.bass_utils import run_bass_kernel_spmd

B, N, F = 4, 2048, 1024
HEAD, HD = 16, 64
NCORES = 8
HG = 2                # head groups (tensor-parallel degree per batch)
FL = F // HG          # local features per core = 512
HL = HEAD // HG       # local heads per core = 8
OC = FL // 128        # o-chunks of 128 in qT/kT = 4
FT = F // 128         # f (contraction) tiles = 8
IC = N // 128         # i/j chunks of 128 = 16
ISPAN = 1024          # attention i-span per inner block
NSP = N // ISPAN      # spans = 2

BF16 = mybir.dt.bfloat16
F32 = mybir.dt.float32
NP_BF16 = mybir.dt.np(BF16)

_CACHED_NC = None
LAST_EXEC_TIME_NS = None
LAST_RES = None


def _build_nc():
    nc = bacc.Bacc("TRN2")

    xT = nc.dram_tensor("xT", [F, N], BF16, kind="ExternalInput")
    wqkT = nc.dram_tensor("wqkT", [F, 2 * FL], BF16, kind="ExternalInput")
    wvT = nc.dram_tensor("wvT", [F, FL], BF16, kind="ExternalInput")
    woT = nc.dram_tensor("woT", [FL, F], BF16, kind="ExternalInput")
    bq = nc.dram_tensor("bq", [FL], F32, kind="ExternalInput")
    bk = nc.dram_tensor("bk", [FL], F32, kind="ExternalInput")
    bv = nc.dram_tensor("bv", [FL], F32, kind="ExternalInput")
    boh = nc.dram_tensor("boh", [F], F32, kind="ExternalInput")
    out = nc.dram_tensor("out", [N, F], F32, kind="ExternalOutput")

    with tile.TileContext(nc) as tc:
        with (
            tc.tile_pool(name="big", bufs=1) as big,
            tc.tile_pool(name="etile", bufs=6) as epool,
            tc.tile_pool(name="auc", bufs=3) as aucp,
            tc.tile_pool(name="ztile", bufs=3) as zpool,
            tc.tile_pool(name="rbc", bufs=3) as rpool,
            tc.tile_pool(name="ostage", bufs=3) as opool,
            tc.tile_pool(name="dspill", bufs=4, space="DRAM") as dpool,
            tc.tile_pool(name="pp", bufs=2, space="PSUM") as pp,
            tc.tile_pool(name="sp", bufs=2, space="PSUM") as spp,
            tc.tile_pool(name="aup", bufs=1, space="PSUM") as aup,
        ):
            # ---- resident SBUF tensors ----
            xT_t = [big.tile([128, N], BF16, tag=f"xT{t}", name=f"xT{t}") for t in range(FT)]
            wqk_sb = big.tile([128, FT, 2 * FL], BF16, tag="wqk")
            wvT_sb = big.tile([128, FT, FL], BF16, tag="wvT")
            woT_sb = big.tile([128, OC, F], BF16, tag="woT")
            qT_oc = [big.tile([128, N], BF16, tag=f"qT{oc}", name=f"qT{oc}") for oc in range(OC)]
            kT_oc = [big.tile([128, N], BF16, tag=f"kT{oc}", name=f"kT{oc}") for oc in range(OC)]
            vAug_ic = [
                big.tile([128, HL, HD + 1], BF16, tag=f"vAug{ic}", name=f"vAug{ic}")
                for ic in range(IC)
            ]
            # pairs 0..2 write full attnT rows; pair 3 is chased by the output
            # projection span by span, so its attnT is split per 512-i span
            attnT_oc = [big.tile([128, N], BF16, tag=f"attnT{oc}", name=f"attnT{oc}") for oc in range(3)]
            attnT3_s = [
                big.tile([128, 512], BF16, tag=f"attnT3s{s}", name=f"attnT3s{s}")
                for s in range(N // 512)
            ]
            bq_sb = big.tile([128, OC], F32, tag="bq")
            bk_sb = big.tile([128, OC], F32, tag="bk")
            bv_sb = big.tile([128, FL], F32, tag="bv")
            bo_sb = big.tile([128, F], F32, tag="bo")

            # ---- input DMAs (small ones first; xT split per f-tile so
            # compute starts as soon as its operands land) ----
            wqkr = wqkT.rearrange("(t p) o -> p t o", p=128)
            nc.sync.dma_start(out=wqk_sb[:, :, FL:], in_=wqkr[:, :, FL:])
            nc.sync.dma_start(out=wqk_sb[:, :, 0:FL], in_=wqkr[:, :, 0:FL])
            xTr = xT.rearrange("(t p) i -> p t i", p=128)
            for t in range(FT):
                nc.sync.dma_start(out=xT_t[t][:], in_=xTr[:, t, :])
            nc.sync.dma_start(
                out=wvT_sb[:], in_=wvT.rearrange("(t p) o -> p t o", p=128)
            )
            nc.sync.dma_start(
                out=bq_sb[:], in_=bq.rearrange("(c p) -> p c", p=128)
            )
            nc.sync.dma_start(
                out=bk_sb[:], in_=bk.rearrange("(c p) -> p c", p=128)
            )
            nc.sync.dma_start(out=bv_sb[:], in_=bv[None, :].partition_broadcast(128))
            nc.sync.dma_start(out=bo_sb[:], in_=boh[None, :].partition_broadcast(128))
            nc.sync.dma_start(
                out=woT_sb[:], in_=woT.rearrange("(t p) g -> p t g", p=128)
            )
            # ones column (64) for every head; V overwrites columns 0..63
            for ic in range(IC):
                nc.vector.memset(vAug_ic[ic][:], 1.0)

            def project_v_chunk(ic):
                ps = pp.tile([128, 512], F32, tag="pp")
                for t in range(FT):
                    nc.tensor.matmul(
                        ps[:],
                        lhsT=xT_t[t][:, ic * 128 : (ic + 1) * 128],
                        rhs=wvT_sb[:, t, :],
                        start=(t == 0),
                        stop=(t == FT - 1),
                    )
                nc.vector.tensor_add(
                    out=vAug_ic[ic][:, :, 0:HD],
                    in0=ps.rearrange("p (h d) -> p h d", h=HL),
                    in1=bv_sb.rearrange("p (h d) -> p h d", h=HL),
                )

            def project_qk(base, b_sb, dst, oc):
                for ic in range(N // 512):
                    ps = pp.tile([128, 512], F32, tag="pp")
                    for t in range(FT):
                        nc.tensor.matmul(
                            ps[:],
                            lhsT=wqk_sb[:, t, base + oc * 128 : base + (oc + 1) * 128],
                            rhs=xT_t[t][:, ic * 512 : (ic + 1) * 512],
                            start=(t == 0),
                            stop=(t == FT - 1),
                        )
                    nc.vector.tensor_scalar_add(
                        out=dst[:, ic * 512 : (ic + 1) * 512],
                        in0=ps[:],
                        scalar1=b_sb[:, oc : oc + 1],
                    )

            def attention_pair(oc, chase=None, per_j0=None):
                # heads h0 = 2*oc (q/k rows 0:64) and h1 = 2*oc+1 (rows 64:128)
                # are processed together: their score matmuls sit on disjoint
                # PE row-groups (K=64 at base partition 0 vs 64) and run
                # concurrently; one [128, 1024] S-PSUM tile holds a 512-wide
                # i-span for each head so exp still works in [128,1024] calls.
                # Score matmuls are emitted one step ahead of the attn-value
                # matmuls so the PE never parks behind an exp-blocked AV and
                # the exp stream stays gapless across span boundaries.
                h0, h1 = 2 * oc, 2 * oc + 1

                def s_emit(isp, j, st):
                    i0 = isp * 512
                    nc.tensor.matmul(
                        st[:, 0:512],
                        lhsT=kT_oc[oc][0:64, j * 128 : (j + 1) * 128],
                        rhs=qT_oc[oc][0:64, i0 : i0 + 512],
                        start=True,
                        stop=True,
                    )
                    nc.tensor.matmul(
                        st[:, 512:1024],
                        lhsT=kT_oc[oc][64:128, j * 128 : (j + 1) * 128],
                        rhs=qT_oc[oc][64:128, i0 : i0 + 512],
                        start=True,
                        stop=True,
                    )

                nxt = spp.tile([128, 1024], F32, tag="sp", name="st")
                s_emit(0, 0, nxt)
                for isp in range(N // 512):
                    if chase is not None:
                        chase(isp)
                    i0 = isp * 512
                    au = aup.tile([HD + 1, 1024], F32, tag="au")
                    for j in range(IC):
                        st = nxt
                        eT = epool.tile([128, 1024], BF16, tag="eT")
                        nc.scalar.activation(
                            eT[:], st[:], mybir.ActivationFunctionType.Exp,
                            scale=1.0 / 32.0,
                        )
                        if j + 1 < IC:
                            nxt = spp.tile([128, 1024], F32, tag="sp", name="st")
                            s_emit(isp, j + 1, nxt)
                        elif isp + 1 < N // 512:
                            nxt = spp.tile([128, 1024], F32, tag="sp", name="st")
                            s_emit(isp + 1, 0, nxt)
                        if isp == 0 and per_j0 is not None:
                            per_j0(j)
                        nc.tensor.matmul(
                            au[:, 0:512],
                            lhsT=vAug_ic[j][:, h0, :],
                            rhs=eT[:, 0:512],
                            start=(j == 0),
                            stop=(j == IC - 1),
                        )
                        nc.tensor.matmul(
                            au[:, 512:1024],
                            lhsT=vAug_ic[j][:, h1, :],
                            rhs=eT[:, 512:1024],
                            start=(j == 0),
                            stop=(j == IC - 1),
                        )
                    # copy attnU + Z out of PSUM promptly to release the bank
                    auc = aucp.tile([HD + 1, 1024], F32, tag="auc")
                    nc.vector.tensor_copy(auc[:], au[:])
                    # 1/Z with decent parallelism: bounce Z through DRAM into
                    # a [128, 8] layout, reciprocal, bounce back broadcast
                    zd = dpool.tile([1, 1024], F32, tag="zd")
                    nc.sync.dma_start(out=zd[:], in_=auc[HD : HD + 1, :])
                    zs = zpool.tile([128, 8], F32, tag="zs")
                    nc.sync.dma_start(
                        out=zs[:], in_=zd[0, :].rearrange("(p f) -> p f", p=128)
                    )
                    zr = zpool.tile([128, 8], F32, tag="zr")
                    nc.vector.reciprocal(zr[:], zs[:])
                    zrd = dpool.tile([1, 1024], F32, tag="zrd")
                    nc.sync.dma_start(
                        out=zrd[0, :].rearrange("(p f) -> p f", p=128), in_=zr[:]
                    )
                    rb = rpool.tile([64, 1024], F32, tag="rb")
                    nc.sync.dma_start(
                        out=rb[:], in_=zrd[0, :].partition_broadcast(64)
                    )
                    if oc < 3:
                        dst0 = attnT_oc[oc][0:64, i0 : i0 + 512]
                        dst1 = attnT_oc[oc][64:128, i0 : i0 + 512]
                    else:
                        dst0 = attnT3_s[isp][0:64, :]
                        dst1 = attnT3_s[isp][64:128, :]
                    nc.vector.tensor_mul(out=dst0, in0=auc[0:HD, 0:512], in1=rb[:, 0:512])
                    nc.vector.tensor_mul(out=dst1, in0=auc[0:HD, 512:1024], in1=rb[:, 512:1024])

            def oproj_span(isp):
                # output projection for i in [isp*512, (isp+1)*512)
                for lic in range(4):
                    ic = isp * 4 + lic
                    for gc in range(F // 512):
                        ps = pp.tile([128, 512], F32, tag="pp")
                        for ct in range(OC):
                            if ct < 3:
                                lhsT = attnT_oc[ct][:, ic * 128 : (ic + 1) * 128]
                            else:
                                lhsT = attnT3_s[isp][:, lic * 128 : (lic + 1) * 128]
                            nc.tensor.matmul(
                                ps[:],
                                lhsT=lhsT,
                                rhs=woT_sb[:, ct, gc * 512 : (gc + 1) * 512],
                                start=(ct == 0),
                                stop=(ct == OC - 1),
                            )
                        st = opool.tile([128, 512], F32, tag="ost")
                        nc.vector.tensor_add(
                            out=st[:], in0=ps[:], in1=bo_sb[:, gc * 512 : (gc + 1) * 512]
                        )
                        nc.sync.dma_start(
                            out=out[ic * 128 : (ic + 1) * 128, gc * 512 : (gc + 1) * 512],
                            in_=st[:],
                        )

            # ---- interleaved projections + attention; o-proj chases pair 3.
            # The v projection is produced just-in-time inside pair 0's first
            # span (attn-value matmul at chunk j only needs v chunk j), so the
            # exp stream starts right after q0/k0 instead of after all of v.
            project_qk(FL, bk_sb, kT_oc[0], 0)
            project_qk(0, bq_sb, qT_oc[0], 0)
            project_v_chunk(0)

            def v_jit(j):
                if j + 1 < IC:
                    project_v_chunk(j + 1)

            attention_pair(0, per_j0=v_jit)
            for oc in range(1, 3):
                project_qk(0, bq_sb, qT_oc[oc], oc)
                project_qk(FL, bk_sb, kT_oc[oc], oc)
                attention_pair(oc)
            project_qk(0, bq_sb, qT_oc[3], 3)
            project_qk(FL, bk_sb, kT_oc[3], 3)
            attention_pair(
                3, chase=lambda isp: oproj_span(isp - 1) if isp >= 1 else None
            )
            oproj_span(N // 512 - 1)

    nc.finalize()
    return nc


def kernel(x, Wq, bq, Wk, bk, Wv, bv, Wo, bo, trace=False):
    global _CACHED_NC, LAST_EXEC_TIME_NS, LAST_RES
    x = np.asarray(x)
    Wq, Wk, Wv, Wo = (np.asarray(a) for a in (Wq, Wk, Wv, Wo))
    bq, bk, bv, bo = (np.asarray(a) for a in (bq, bk, bv, bo))

    if _CACHED_NC is None:
        _CACHED_NC = _build_nc()
    nc = _CACHED_NC

    # host-side shard prep (transposes + bf16 casts)
    xT_b = [np.ascontiguousarray(x[b].T).astype(NP_BF16) for b in range(B)]
    wqkT_g = [
        np.ascontiguousarray(
            np.concatenate(
                [Wq[g * FL : (g + 1) * FL, :].T, Wk[g * FL : (g + 1) * FL, :].T],
                axis=1,
            )
        ).astype(NP_BF16)
        for g in range(HG)
    ]
    wvT_g = [np.ascontiguousarray(Wv[g * FL : (g + 1) * FL, :].T).astype(NP_BF16) for g in range(HG)]
    woT_g = [np.ascontiguousarray(Wo[:, g * FL : (g + 1) * FL].T).astype(NP_BF16) for g in range(HG)]
    bq_g = [np.ascontiguousarray(bq[g * FL : (g + 1) * FL]).astype(np.float32) for g in range(HG)]
    bk_g = [np.ascontiguousarray(bk[g * FL : (g + 1) * FL]).astype(np.float32) for g in range(HG)]
    bv_g = [np.ascontiguousarray(bv[g * FL : (g + 1) * FL]).astype(np.float32) for g in range(HG)]
    bo_half = (bo.astype(np.float32) / 2.0)

    in_maps = []
    for c in range(NCORES):
        b, g = c // HG, c % HG
        in_maps.append(
            {
                "xT": xT_b[b],
                "wqkT": wqkT_g[g],
                "wvT": wvT_g[g],
                "woT": woT_g[g],
                "bq": bq_g[g],
                "bk": bk_g[g],
                "bv": bv_g[g],
                "boh": bo_half,
            }
        )

    res = run_bass_kernel_spmd(nc, in_maps, core_ids=list(range(NCORES)), trace=trace)
    LAST_EXEC_TIME_NS = res.exec_time_ns
    LAST_RES = res

    out = np.empty((B, N, F), np.float32)
    for b in range(B):
        out[b] = res.results[2 * b]["out"] + res.results[2 * b + 1]["out"]
    return out


# revision 22
# speedup vs baseline: 1.0200x; 1.0200x over previous
"""Multi-head self-attention (B=4, N=2048, F=1024, 16 heads) on 8 TRN2 NeuronCores.

Sharding (Megatron-style, per the hint): data-parallel over the 4 batches x
tensor-parallel split of the 16 heads into 2 groups of 8. Core c handles
batch c//2 with head group c%2 (512 of the 1024 qkv features, column-split
Wq/Wk/Wv, row-split Wo). Each core emits a partial o-projection [2048, 1024];
the host unshard sums the pair of partials per batch (the Megatron
all-reduce) and stacks batches.

Device kernel layout choices (all matmuls bf16 with f32 PSUM accumulation):
  xT   [1024f, 2048i]  (x transposed on host)  - rhs for q/k, lhsT for v
  qT/kT [512o, 2048i]  (o = head-major features, on partitions)
  vAug [2048j, 8h, 65] (per head: V columns 0..63 plus a ones column at 64
                        so the attention-value matmul also yields the softmax
                        denominator Z as output row 64)
  scores S^T [j, i] via lhsT=kT-chunk, rhs=qT; exp on ScalarE (scale=1/32,
  no max subtraction needed: |S/32| <~ 1.5); attnU^T accumulated over j in
  PSUM, copied to SBUF promptly to release the PSUM bank; softmax
  normalization (1/Z broadcast) runs off the critical path via small DMA
  round-trips through DRAM.

QKV is interleaved with attention per 128-row chunk of q/k so the ScalarE
exp stream (the steady-state bottleneck) starts early and the remaining
projections hide under it.
"""

import sys
import types

sys.path.insert(0, "/opt/trn_rl_repo")

import numpy as np

# Best-effort: register the axon NTFF profile hook so trace=True works
# (used by test harnesses via BASS_TRACE); harmless when tracing is off.
try:
    import antenv

    if "antenv.axon_hooks" not in sys.modules:
        from trn_agent_boot.trn_boot import _ntff_profile_via_ctypes

        _hooks_mod = types.ModuleType("antenv.axon_hooks")
        _hook = _ntff_profile_via_ctypes("/opt/axon/libaxon_pjrt.so")
        _hooks_mod.get_axon_ntff_profile_hook = lambda: _hook
        _hooks_mod.set_axon_ntff_profile_hook = lambda h: None
        sys.modules["antenv.axon_hooks"] = _hooks_mod
        antenv.axon_hooks = _hooks_mod
except Exception:
    pass

import concourse.bacc as bacc
import concourse.tile as tile
from concourse import mybir
from concourse.bass_utils import run_bass_kernel_spmd

B, N, F = 4, 2048, 1024
HEAD, HD = 16, 64
NCORES = 8
HG = 2                # head groups (tensor-parallel degree per batch)
FL = F // HG          # local features per core = 512
HL = HEAD // HG       # local heads per core = 8
OC = FL // 128        # o-chunks of 128 in qT/kT = 4
FT = F // 128         # f (contraction) tiles = 8
IC = N // 128         # i/j chunks of 128 = 16
ISPAN = 1024          # attention i-span per inner block
NSP = N // ISPAN      # spans = 2

BF16 = mybir.dt.bfloat16
F32 = mybir.dt.float32
NP_BF16 = mybir.dt.np(BF16)

_CACHED_NC = None
LAST_EXEC_TIME_NS = None
LAST_RES = None


def _build_nc():
    nc = bacc.Bacc("TRN2")

    xT = nc.dram_tensor("xT", [F, N], BF16, kind="ExternalInput")
    wqkT = nc.dram_tensor("wqkT", [F, 2 * FL], BF16, kind="ExternalInput")
    wvT = nc.dram_tensor("wvT", [F, FL], BF16, kind="ExternalInput")
    woT = nc.dram_tensor("woT", [FL, F], BF16, kind="ExternalInput")
    bq = nc.dram_tensor("bq", [FL], F32, kind="ExternalInput")
    bk = nc.dram_tensor("bk", [FL], F32, kind="ExternalInput")
    bv = nc.dram_tensor("bv", [FL], F32, kind="ExternalInput")
    boh = nc.dram_tensor("boh", [F], F32, kind="ExternalInput")
    out = nc.dram_tensor("out", [N, F], F32, kind="ExternalOutput")

    with tile.TileContext(nc) as tc:
        with (
            tc.tile_pool(name="big", bufs=1) as big,
            tc.tile_pool(name="etile", bufs=6) as epool,
            tc.tile_pool(name="auc", bufs=3) as aucp,
            tc.tile_pool(name="ztile", bufs=3) as zpool,
            tc.tile_pool(name="rbc", bufs=3) as rpool,
            tc.tile_pool(name="ostage", bufs=3) as opool,
            tc.tile_pool(name="dspill", bufs=4, space="DRAM") as dpool,
            tc.tile_pool(name="pp", bufs=2, space="PSUM") as pp,
            tc.tile_pool(name="sp", bufs=2, space="PSUM") as spp,
            tc.tile_pool(name="aup", bufs=1, space="PSUM") as aup,
        ):
            # ---- resident SBUF tensors ----
            xT_t = [big.tile([128, N], BF16, tag=f"xT{t}", name=f"xT{t}") for t in range(FT)]
            wqk_sb = big.tile([128, FT, 2 * FL], BF16, tag="wqk")
            wvT_sb = big.tile([128, FT, FL], BF16, tag="wvT")
            woT_sb = big.tile([128, OC, F], BF16, tag="woT")
            qT_oc = [big.tile([128, N], BF16, tag=f"qT{oc}", name=f"qT{oc}") for oc in range(OC)]
            kT_oc = [big.tile([128, N], BF16, tag=f"kT{oc}", name=f"kT{oc}") for oc in range(OC)]
            vAug_ic = [
                big.tile([128, HL, HD + 1], BF16, tag=f"vAug{ic}", name=f"vAug{ic}")
                for ic in range(IC)
            ]
            # pairs 0..2 write full attnT rows; pair 3 is chased by the output
            # projection span by span, so its attnT is split per 512-i span
            attnT_oc = [big.tile([128, N], BF16, tag=f"attnT{oc}", name=f"attnT{oc}") for oc in range(3)]
            attnT3_s = [
                big.tile([128, 512], BF16, tag=f"attnT3s{s}", name=f"attnT3s{s}")
                for s in range(N // 512)
            ]
            bq_sb = big.tile([128, OC], F32, tag="bq")
            bk_sb = big.tile([128, OC], F32, tag="bk")
            bv_sb = big.tile([128, FL], F32, tag="bv")
            bo_sb = big.tile([128, F], F32, tag="bo")

            # ---- input DMAs (small ones first; xT split per f-tile so
            # compute starts as soon as its operands land) ----
            wqkr = wqkT.rearrange("(t p) o -> p t o", p=128)
            nc.sync.dma_start(out=wqk_sb[:, :, FL:], in_=wqkr[:, :, FL:])
            nc.sync.dma_start(out=wqk_sb[:, :, 0:FL], in_=wqkr[:, :, 0:FL])
            xTr = xT.rearrange("(t p) i -> p t i", p=128)
            for t in range(FT):
                nc.sync.dma_start(out=xT_t[t][:], in_=xTr[:, t, :])
            nc.sync.dma_start(
                out=wvT_sb[:], in_=wvT.rearrange("(t p) o -> p t o", p=128)
            )
            nc.sync.dma_start(
                out=bq_sb[:], in_=bq.rearrange("(c p) -> p c", p=128)
            )
            nc.sync.dma_start(
                out=bk_sb[:], in_=bk.rearrange("(c p) -> p c", p=128)
            )
            nc.sync.dma_start(out=bv_sb[:], in_=bv[None, :].partition_broadcast(128))
            nc.sync.dma_start(out=bo_sb[:], in_=boh[None, :].partition_broadcast(128))
            nc.sync.dma_start(
                out=woT_sb[:], in_=woT.rearrange("(t p) g -> p t g", p=128)
            )
            # ones column (64) for every head; V overwrites columns 0..63
            for ic in range(IC):
                nc.vector.memset(vAug_ic[ic][:], 1.0)

            def project_v_chunk(ic):
                ps = pp.tile([128, 512], F32, tag="pp")
                for t in range(FT):
                    nc.tensor.matmul(
                        ps[:],
                        lhsT=xT_t[t][:, ic * 128 : (ic + 1) * 128],
                        rhs=wvT_sb[:, t, :],
                        start=(t == 0),
                        stop=(t == FT - 1),
                    )
                nc.vector.tensor_add(
                    out=vAug_ic[ic][:, :, 0:HD],
                    in0=ps.rearrange("p (h d) -> p h d", h=HL),
                    in1=bv_sb.rearrange("p (h d) -> p h d", h=HL),
                )

            def project_qk(base, b_sb, dst, oc):
                for ic in range(N // 512):
                    ps = pp.tile([128, 512], F32, tag="pp")
                    for t in range(FT):
                        nc.tensor.matmul(
                            ps[:],
                            lhsT=wqk_sb[:, t, base + oc * 128 : base + (oc + 1) * 128],
                            rhs=xT_t[t][:, ic * 512 : (ic + 1) * 512],
                            start=(t == 0),
                            stop=(t == FT - 1),
                        )
                    nc.vector.tensor_scalar_add(
                        out=dst[:, ic * 512 : (ic + 1) * 512],
                        in0=ps[:],
                        scalar1=b_sb[:, oc : oc + 1],
                    )

            def attention_pair(oc, chase=None, per_j0=None):
                # heads h0 = 2*oc (q/k rows 0:64) and h1 = 2*oc+1 (rows 64:128)
                # are processed together: their score matmuls sit on disjoint
                # PE row-groups (K=64 at base partition 0 vs 64) and run
                # concurrently; one [128, 1024] S-PSUM tile holds a 512-wide
                # i-span for each head so exp still works in [128,1024] calls.
                # Score matmuls are emitted one step ahead of the attn-value
                # matmuls so the PE never parks behind an exp-blocked AV and
                # the exp stream stays gapless across span boundaries.
                h0, h1 = 2 * oc, 2 * oc + 1

                def s_emit(isp, j, st):
                    i0 = isp * 512
                    nc.tensor.matmul(
                        st[:, 0:512],
                        lhsT=kT_oc[oc][0:64, j * 128 : (j + 1) * 128],
                        rhs=qT_oc[oc][0:64, i0 : i0 + 512],
                        start=True,
                        stop=True,
                    )
                    nc.tensor.matmul(
                        st[:, 512:1024],
                        lhsT=kT_oc[oc][64:128, j * 128 : (j + 1) * 128],
                        rhs=qT_oc[oc][64:128, i0 : i0 + 512],
                        start=True,
                        stop=True,
                    )

                nxt = spp.tile([128, 1024], F32, tag="sp", name="st")
                s_emit(0, 0, nxt)
                for isp in range(N // 512):
                    i0 = isp * 512
                    au = aup.tile([HD + 1, 1024], F32, tag="au")
                    for j in range(IC):
                        st = nxt
                        eT = epool.tile([128, 1024], BF16, tag="eT")
                        nc.scalar.activation(
                            eT[:], st[:], mybir.ActivationFunctionType.Exp,
                            scale=1.0 / 32.0,
                        )
                        if j + 1 < IC:
                            nxt = spp.tile([128, 1024], F32, tag="sp", name="st")
                            s_emit(isp, j + 1, nxt)
                        elif isp + 1 < N // 512:
                            nxt = spp.tile([128, 1024], F32, tag="sp", name="st")
                            s_emit(isp + 1, 0, nxt)
                        if isp == 0 and per_j0 is not None:
                            per_j0(j)
                        nc.tensor.matmul(
                            au[:, 0:512],
                            lhsT=vAug_ic[j][:, h0, :],
                            rhs=eT[:, 0:512],
                            start=(j == 0),
                            stop=(j == IC - 1),
                        )
                        nc.tensor.matmul(
                            au[:, 512:1024],
                            lhsT=vAug_ic[j][:, h1, :],
                            rhs=eT[:, 512:1024],
                            start=(j == 0),
                            stop=(j == IC - 1),
                        )
                    # copy attnU + Z out of PSUM promptly to release the bank
                    auc = aucp.tile([HD + 1, 1024], F32, tag="auc")
                    nc.vector.tensor_copy(auc[:], au[:])
                    # 1/Z with decent parallelism: bounce Z through DRAM into
                    # a [128, 8] layout, reciprocal, bounce back broadcast
                    zd = dpool.tile([1, 1024], F32, tag="zd")
                    nc.sync.dma_start(out=zd[:], in_=auc[HD : HD + 1, :])
                    zs = zpool.tile([128, 8], F32, tag="zs")
                    nc.sync.dma_start(
                        out=zs[:], in_=zd[0, :].rearrange("(p f) -> p f", p=128)
                    )
                    zr = zpool.tile([128, 8], F32, tag="zr")
                    nc.vector.reciprocal(zr[:], zs[:])
                    zrd = dpool.tile([1, 1024], F32, tag="zrd")
                    nc.sync.dma_start(
                        out=zrd[0, :].rearrange("(p f) -> p f", p=128), in_=zr[:]
                    )
                    rb = rpool.tile([64, 1024], F32, tag="rb")
                    nc.sync.dma_start(
                        out=rb[:], in_=zrd[0, :].partition_broadcast(64)
                    )
                    if oc < 3:
                        dst0 = attnT_oc[oc][0:64, i0 : i0 + 512]
                        dst1 = attnT_oc[oc][64:128, i0 : i0 + 512]
                    else:
                        dst0 = attnT3_s[isp][0:64, :]
                        dst1 = attnT3_s[isp][64:128, :]
                    nc.vector.tensor_mul(out=dst0, in0=auc[0:HD, 0:512], in1=rb[:, 0:512])
                    nc.vector.tensor_mul(out=dst1, in0=auc[0:HD, 512:1024], in1=rb[:, 512:1024])
                    if chase is not None:
                        chase(isp)

            def oproj_span(isp):
                # output projection for i in [isp*512, (isp+1)*512)
                for lic in range(4):
                    ic = isp * 4 + lic
                    for gc in range(F // 512):
                        ps = pp.tile([128, 512], F32, tag="pp")
                        for ct in range(OC):
                            if ct < 3:
                                lhsT = attnT_oc[ct][:, ic * 128 : (ic + 1) * 128]
                            else:
                                lhsT = attnT3_s[isp][:, lic * 128 : (lic + 1) * 128]
                            nc.tensor.matmul(
                                ps[:],
                                lhsT=lhsT,
                                rhs=woT_sb[:, ct, gc * 512 : (gc + 1) * 512],
                                start=(ct == 0),
                                stop=(ct == OC - 1),
                            )
                        st = opool.tile([128, 512], F32, tag="ost")
                        nc.vector.tensor_add(
                            out=st[:], in0=ps[:], in1=bo_sb[:, gc * 512 : (gc + 1) * 512]
                        )
                        nc.sync.dma_start(
                            out=out[ic * 128 : (ic + 1) * 128, gc * 512 : (gc + 1) * 512],
                            in_=st[:],
                        )

            # ---- interleaved projections + attention; o-proj chases pair 3.
            # The v projection is produced just-in-time inside pair 0's first
            # span (attn-value matmul at chunk j only needs v chunk j), so the
            # exp stream starts right after q0/k0 instead of after all of v.
            project_qk(FL, bk_sb, kT_oc[0], 0)
            project_qk(0, bq_sb, qT_oc[0], 0)
            project_v_chunk(0)

            def v_jit(j):
                if j + 1 < IC:
                    project_v_chunk(j + 1)

            attention_pair(0, per_j0=v_jit)
            for oc in range(1, 3):
                project_qk(0, bq_sb, qT_oc[oc], oc)
                project_qk(FL, bk_sb, kT_oc[oc], oc)
                attention_pair(oc)
            project_qk(0, bq_sb, qT_oc[3], 3)
            project_qk(FL, bk_sb, kT_oc[3], 3)
            attention_pair(
                3, chase=lambda isp: oproj_span(isp - 1) if isp >= 1 else None
            )
            oproj_span(N // 512 - 1)

    nc.finalize()
    return nc


def kernel(x, Wq, bq, Wk, bk, Wv, bv, Wo, bo, trace=False):
    global _CACHED_NC, LAST_EXEC_TIME_NS, LAST_RES
    x = np.asarray(x)
    Wq, Wk, Wv, Wo = (np.asarray(a) for a in (Wq, Wk, Wv, Wo))
    bq, bk, bv, bo = (np.asarray(a) for a in (bq, bk, bv, bo))

    if _CACHED_NC is None:
        _CACHED_NC = _build_nc()
    nc = _CACHED_NC

    # host-side shard prep (transposes + bf16 casts)
    xT_b = [np.ascontiguousarray(x[b].T).astype(NP_BF16) for b in range(B)]
    wqkT_g = [
        np.ascontiguousarray(
            np.concatenate(
                [Wq[g * FL : (g + 1) * FL, :].T, Wk[g * FL : (g + 1) * FL, :].T],
                axis=1,
            )
        ).astype(NP_BF16)
        for g in range(HG)
    ]
    wvT_g = [np.ascontiguousarray(Wv[g * FL : (g + 1) * FL, :].T).astype(NP_BF16) for g in range(HG)]
    woT_g = [np.ascontiguousarray(Wo[:, g * FL : (g + 1) * FL].T).astype(NP_BF16) for g in range(HG)]
    bq_g = [np.ascontiguousarray(bq[g * FL : (g + 1) * FL]).astype(np.float32) for g in range(HG)]
    bk_g = [np.ascontiguousarray(bk[g * FL : (g + 1) * FL]).astype(np.float32) for g in range(HG)]
    bv_g = [np.ascontiguousarray(bv[g * FL : (g + 1) * FL]).astype(np.float32) for g in range(HG)]
    bo_half = (bo.astype(np.float32) / 2.0)

    in_maps = []
    for c in range(NCORES):
        b, g = c // HG, c % HG
        in_maps.append(
            {
                "xT": xT_b[b],
                "wqkT": wqkT_g[g],
                "wvT": wvT_g[g],
                "woT": woT_g[g],
                "bq": bq_g[g],
                "bk": bk_g[g],
                "bv": bv_g[g],
                "boh": bo_half,
            }
        )

    res = run_bass_kernel_spmd(nc, in_maps, core_ids=list(range(NCORES)), trace=trace)
    LAST_EXEC_TIME_NS = res.exec_time_ns
    LAST_RES = res

    out = np.empty((B, N, F), np.float32)
    for b in range(B):
        out[b] = res.results[2 * b]["out"] + res.results[2 * b + 1]["out"]
    return out


# revision 23
# speedup vs baseline: 1.0318x; 1.0116x over previous
"""Multi-head self-attention (B=4, N=2048, F=1024, 16 heads) on 8 TRN2 NeuronCores.

Sharding (Megatron-style, per the hint): data-parallel over the 4 batches x
tensor-parallel split of the 16 heads into 2 groups of 8. Core c handles
batch c//2 with head group c%2 (512 of the 1024 qkv features, column-split
Wq/Wk/Wv, row-split Wo). Each core emits a partial o-projection [2048, 1024];
the host unshard sums the pair of partials per batch (the Megatron
all-reduce) and stacks batches.

Device kernel layout choices (all matmuls bf16 with f32 PSUM accumulation):
  xT   [1024f, 2048i]  (x transposed on host)  - rhs for q/k, lhsT for v
  qT/kT [512o, 2048i]  (o = head-major features, on partitions)
  vAug [2048j, 8h, 65] (per head: V columns 0..63 plus a ones column at 64
                        so the attention-value matmul also yields the softmax
                        denominator Z as output row 64)
  scores S^T [j, i] via lhsT=kT-chunk, rhs=qT; exp on ScalarE (scale=1/32,
  no max subtraction needed: |S/32| <~ 1.5); attnU^T accumulated over j in
  PSUM, copied to SBUF promptly to release the PSUM bank; softmax
  normalization (1/Z broadcast) runs off the critical path via small DMA
  round-trips through DRAM.

QKV is interleaved with attention per 128-row chunk of q/k so the ScalarE
exp stream (the steady-state bottleneck) starts early and the remaining
projections hide under it.
"""

import sys
import types

sys.path.insert(0, "/opt/trn_rl_repo")

import numpy as np

# Best-effort: register the axon NTFF profile hook so trace=True works
# (used by test harnesses via BASS_TRACE); harmless when tracing is off.
try:
    import antenv

    if "antenv.axon_hooks" not in sys.modules:
        from trn_agent_boot.trn_boot import _ntff_profile_via_ctypes

        _hooks_mod = types.ModuleType("antenv.axon_hooks")
        _hook = _ntff_profile_via_ctypes("/opt/axon/libaxon_pjrt.so")
        _hooks_mod.get_axon_ntff_profile_hook = lambda: _hook
        _hooks_mod.set_axon_ntff_profile_hook = lambda h: None
        sys.modules["antenv.axon_hooks"] = _hooks_mod
        antenv.axon_hooks = _hooks_mod
except Exception:
    pass

import concourse.bacc as bacc
import concourse.tile as tile
from concourse import mybir
from concourse.bass_utils import run_bass_kernel_spmd

B, N, F = 4, 2048, 1024
HEAD, HD = 16, 64
NCORES = 8
HG = 2                # head groups (tensor-parallel degree per batch)
FL = F // HG          # local features per core = 512
HL = HEAD // HG       # local heads per core = 8
OC = FL // 128        # o-chunks of 128 in qT/kT = 4
FT = F // 128         # f (contraction) tiles = 8
IC = N // 128         # i/j chunks of 128 = 16
ISPAN = 1024          # attention i-span per inner block
NSP = N // ISPAN      # spans = 2

BF16 = mybir.dt.bfloat16
F32 = mybir.dt.float32
NP_BF16 = mybir.dt.np(BF16)

_CACHED_NC = None
LAST_EXEC_TIME_NS = None
LAST_RES = None


def _build_nc():
    nc = bacc.Bacc("TRN2")

    xT = nc.dram_tensor("xT", [F, N], BF16, kind="ExternalInput")
    wqkT = nc.dram_tensor("wqkT", [F, 2 * FL], BF16, kind="ExternalInput")
    wvT = nc.dram_tensor("wvT", [F, FL], BF16, kind="ExternalInput")
    woT = nc.dram_tensor("woT", [FL, F], BF16, kind="ExternalInput")
    bq = nc.dram_tensor("bq", [FL], F32, kind="ExternalInput")
    bk = nc.dram_tensor("bk", [FL], F32, kind="ExternalInput")
    bv = nc.dram_tensor("bv", [FL], F32, kind="ExternalInput")
    boh = nc.dram_tensor("boh", [F], F32, kind="ExternalInput")
    out = nc.dram_tensor("out", [N, F], F32, kind="ExternalOutput")

    with tile.TileContext(nc) as tc:
        with (
            tc.tile_pool(name="big", bufs=1) as big,
            tc.tile_pool(name="etile", bufs=6) as epool,
            tc.tile_pool(name="auc", bufs=3) as aucp,
            tc.tile_pool(name="ztile", bufs=3) as zpool,
            tc.tile_pool(name="rbc", bufs=3) as rpool,
            tc.tile_pool(name="ostage", bufs=3) as opool,
            tc.tile_pool(name="dspill", bufs=4, space="DRAM") as dpool,
            tc.tile_pool(name="pp", bufs=2, space="PSUM") as pp,
            tc.tile_pool(name="sp", bufs=2, space="PSUM") as spp,
            tc.tile_pool(name="aup", bufs=1, space="PSUM") as aup,
        ):
            # ---- resident SBUF tensors ----
            xT_t = [big.tile([128, N], BF16, tag=f"xT{t}", name=f"xT{t}") for t in range(FT)]
            wqk_sb = big.tile([128, FT, 2 * FL], BF16, tag="wqk")
            wvT_sb = big.tile([128, FT, FL], BF16, tag="wvT")
            woT_sb = big.tile([128, OC, F], BF16, tag="woT")
            qT_oc = [big.tile([128, N], BF16, tag=f"qT{oc}", name=f"qT{oc}") for oc in range(OC)]
            kT_oc = [big.tile([128, N], BF16, tag=f"kT{oc}", name=f"kT{oc}") for oc in range(OC)]
            vAug_ic = [
                big.tile([128, HL, HD + 1], BF16, tag=f"vAug{ic}", name=f"vAug{ic}")
                for ic in range(IC)
            ]
            # pairs 0..2 write full attnT rows; pair 3 is chased by the output
            # projection span by span, so its attnT is split per 512-i span
            attnT_oc = [big.tile([128, N], BF16, tag=f"attnT{oc}", name=f"attnT{oc}") for oc in range(3)]
            attnT3_s = [
                big.tile([128, 512], BF16, tag=f"attnT3s{s}", name=f"attnT3s{s}")
                for s in range(N // 512)
            ]
            bq_sb = big.tile([128, OC], F32, tag="bq")
            bk_sb = big.tile([128, OC], F32, tag="bk")
            bv_sb = big.tile([128, FL], F32, tag="bv")
            bo_sb = big.tile([128, F], F32, tag="bo")

            # ---- input DMAs (small ones first; xT split per f-tile so
            # compute starts as soon as its operands land) ----
            wqkr = wqkT.rearrange("(t p) o -> p t o", p=128)
            nc.sync.dma_start(out=wqk_sb[:, :, FL:], in_=wqkr[:, :, FL:])
            nc.sync.dma_start(out=wqk_sb[:, :, 0:FL], in_=wqkr[:, :, 0:FL])
            xTr = xT.rearrange("(t p) i -> p t i", p=128)
            for t in range(FT):
                nc.sync.dma_start(out=xT_t[t][:], in_=xTr[:, t, :])
            nc.sync.dma_start(
                out=wvT_sb[:], in_=wvT.rearrange("(t p) o -> p t o", p=128)
            )
            nc.sync.dma_start(
                out=bq_sb[:], in_=bq.rearrange("(c p) -> p c", p=128)
            )
            nc.sync.dma_start(
                out=bk_sb[:], in_=bk.rearrange("(c p) -> p c", p=128)
            )
            nc.sync.dma_start(out=bv_sb[:], in_=bv[None, :].partition_broadcast(128))
            nc.sync.dma_start(out=bo_sb[:], in_=boh[None, :].partition_broadcast(128))
            nc.sync.dma_start(
                out=woT_sb[:], in_=woT.rearrange("(t p) g -> p t g", p=128)
            )
            # ones column (64) for every head; V overwrites columns 0..63
            for ic in range(IC):
                nc.vector.memset(vAug_ic[ic][:], 1.0)

            # PE warmup: dummy matmuls while the input DMAs are in flight so
            # the HAM clock-gate reaches 2.4 GHz before the real work starts
            # (otherwise the first ~3.4us of projections run at half clock).
            wup = big.tile([128, 128], BF16, tag="wup")
            nc.vector.memset(wup[:], 0.0)
            wps = pp.tile([128, 512], F32, tag="pp", name="wps")
            for w in range(72):
                nc.tensor.matmul(
                    wps[:, 0:128],
                    lhsT=wup[:],
                    rhs=wup[:],
                    start=True,
                    stop=True,
                )

            def project_v_chunk(ic):
                ps = pp.tile([128, 512], F32, tag="pp")
                for t in range(FT):
                    nc.tensor.matmul(
                        ps[:],
                        lhsT=xT_t[t][:, ic * 128 : (ic + 1) * 128],
                        rhs=wvT_sb[:, t, :],
                        start=(t == 0),
                        stop=(t == FT - 1),
                    )
                nc.vector.tensor_add(
                    out=vAug_ic[ic][:, :, 0:HD],
                    in0=ps.rearrange("p (h d) -> p h d", h=HL),
                    in1=bv_sb.rearrange("p (h d) -> p h d", h=HL),
                )

            def project_qk(base, b_sb, dst, oc):
                for ic in range(N // 512):
                    ps = pp.tile([128, 512], F32, tag="pp")
                    for t in range(FT):
                        nc.tensor.matmul(
                            ps[:],
                            lhsT=wqk_sb[:, t, base + oc * 128 : base + (oc + 1) * 128],
                            rhs=xT_t[t][:, ic * 512 : (ic + 1) * 512],
                            start=(t == 0),
                            stop=(t == FT - 1),
                        )
                    nc.vector.tensor_scalar_add(
                        out=dst[:, ic * 512 : (ic + 1) * 512],
                        in0=ps[:],
                        scalar1=b_sb[:, oc : oc + 1],
                    )

            def attention_pair(oc, chase=None, per_j0=None):
                # heads h0 = 2*oc (q/k rows 0:64) and h1 = 2*oc+1 (rows 64:128)
                # are processed together: their score matmuls sit on disjoint
                # PE row-groups (K=64 at base partition 0 vs 64) and run
                # concurrently; one [128, 1024] S-PSUM tile holds a 512-wide
                # i-span for each head so exp still works in [128,1024] calls.
                # Score matmuls are emitted one step ahead of the attn-value
                # matmuls so the PE never parks behind an exp-blocked AV and
                # the exp stream stays gapless across span boundaries.
                h0, h1 = 2 * oc, 2 * oc + 1

                def s_emit(isp, j, st):
                    i0 = isp * 512
                    nc.tensor.matmul(
                        st[:, 0:512],
                        lhsT=kT_oc[oc][0:64, j * 128 : (j + 1) * 128],
                        rhs=qT_oc[oc][0:64, i0 : i0 + 512],
                        start=True,
                        stop=True,
                    )
                    nc.tensor.matmul(
                        st[:, 512:1024],
                        lhsT=kT_oc[oc][64:128, j * 128 : (j + 1) * 128],
                        rhs=qT_oc[oc][64:128, i0 : i0 + 512],
                        start=True,
                        stop=True,
                    )

                nxt = spp.tile([128, 1024], F32, tag="sp", name="st")
                s_emit(0, 0, nxt)
                for isp in range(N // 512):
                    i0 = isp * 512
                    au = aup.tile([HD + 1, 1024], F32, tag="au")
                    for j in range(IC):
                        st = nxt
                        eT = epool.tile([128, 1024], BF16, tag="eT")
                        nc.scalar.activation(
                            eT[:], st[:], mybir.ActivationFunctionType.Exp,
                            scale=1.0 / 32.0,
                        )
                        if j + 1 < IC:
                            nxt = spp.tile([128, 1024], F32, tag="sp", name="st")
                            s_emit(isp, j + 1, nxt)
                        elif isp + 1 < N // 512:
                            nxt = spp.tile([128, 1024], F32, tag="sp", name="st")
                            s_emit(isp + 1, 0, nxt)
                        if isp == 0 and per_j0 is not None:
                            per_j0(j)
                        nc.tensor.matmul(
                            au[:, 0:512],
                            lhsT=vAug_ic[j][:, h0, :],
                            rhs=eT[:, 0:512],
                            start=(j == 0),
                            stop=(j == IC - 1),
                        )
                        nc.tensor.matmul(
                            au[:, 512:1024],
                            lhsT=vAug_ic[j][:, h1, :],
                            rhs=eT[:, 512:1024],
                            start=(j == 0),
                            stop=(j == IC - 1),
                        )
                    # copy attnU + Z out of PSUM promptly to release the bank
                    auc = aucp.tile([HD + 1, 1024], F32, tag="auc")
                    nc.vector.tensor_copy(auc[:], au[:])
                    # 1/Z with decent parallelism: bounce Z through DRAM into
                    # a [128, 8] layout, reciprocal, bounce back broadcast
                    zd = dpool.tile([1, 1024], F32, tag="zd")
                    nc.sync.dma_start(out=zd[:], in_=auc[HD : HD + 1, :])
                    zs = zpool.tile([128, 8], F32, tag="zs")
                    nc.sync.dma_start(
                        out=zs[:], in_=zd[0, :].rearrange("(p f) -> p f", p=128)
                    )
                    zr = zpool.tile([128, 8], F32, tag="zr")
                    nc.vector.reciprocal(zr[:], zs[:])
                    zrd = dpool.tile([1, 1024], F32, tag="zrd")
                    nc.sync.dma_start(
                        out=zrd[0, :].rearrange("(p f) -> p f", p=128), in_=zr[:]
                    )
                    rb = rpool.tile([64, 1024], F32, tag="rb")
                    nc.sync.dma_start(
                        out=rb[:], in_=zrd[0, :].partition_broadcast(64)
                    )
                    if oc < 3:
                        dst0 = attnT_oc[oc][0:64, i0 : i0 + 512]
                        dst1 = attnT_oc[oc][64:128, i0 : i0 + 512]
                    else:
                        dst0 = attnT3_s[isp][0:64, :]
                        dst1 = attnT3_s[isp][64:128, :]
                    nc.vector.tensor_mul(out=dst0, in0=auc[0:HD, 0:512], in1=rb[:, 0:512])
                    nc.vector.tensor_mul(out=dst1, in0=auc[0:HD, 512:1024], in1=rb[:, 512:1024])
                    if chase is not None:
                        chase(isp)

            def oproj_span(isp):
                # output projection for i in [isp*512, (isp+1)*512)
                for lic in range(4):
                    ic = isp * 4 + lic
                    for gc in range(F // 512):
                        ps = pp.tile([128, 512], F32, tag="pp")
                        for ct in range(OC):
                            if ct < 3:
                                lhsT = attnT_oc[ct][:, ic * 128 : (ic + 1) * 128]
                            else:
                                lhsT = attnT3_s[isp][:, lic * 128 : (lic + 1) * 128]
                            nc.tensor.matmul(
                                ps[:],
                                lhsT=lhsT,
                                rhs=woT_sb[:, ct, gc * 512 : (gc + 1) * 512],
                                start=(ct == 0),
                                stop=(ct == OC - 1),
                            )
                        st = opool.tile([128, 512], F32, tag="ost")
                        nc.vector.tensor_add(
                            out=st[:], in0=ps[:], in1=bo_sb[:, gc * 512 : (gc + 1) * 512]
                        )
                        nc.sync.dma_start(
                            out=out[ic * 128 : (ic + 1) * 128, gc * 512 : (gc + 1) * 512],
                            in_=st[:],
                        )

            # ---- interleaved projections + attention; o-proj chases pair 3.
            # The v projection is produced just-in-time inside pair 0's first
            # span (attn-value matmul at chunk j only needs v chunk j), so the
            # exp stream starts right after q0/k0 instead of after all of v.
            project_qk(FL, bk_sb, kT_oc[0], 0)
            project_qk(0, bq_sb, qT_oc[0], 0)
            project_v_chunk(0)

            def v_jit(j):
                if j + 1 < IC:
                    project_v_chunk(j + 1)

            attention_pair(0, per_j0=v_jit)
            for oc in range(1, 3):
                project_qk(0, bq_sb, qT_oc[oc], oc)
                project_qk(FL, bk_sb, kT_oc[oc], oc)
                attention_pair(oc)
            project_qk(0, bq_sb, qT_oc[3], 3)
            project_qk(FL, bk_sb, kT_oc[3], 3)
            attention_pair(
                3, chase=lambda isp: oproj_span(isp - 1) if isp >= 1 else None
            )
            oproj_span(N // 512 - 1)

    nc.finalize()
    return nc


def kernel(x, Wq, bq, Wk, bk, Wv, bv, Wo, bo, trace=False):
    global _CACHED_NC, LAST_EXEC_TIME_NS, LAST_RES
    x = np.asarray(x)
    Wq, Wk, Wv, Wo = (np.asarray(a) for a in (Wq, Wk, Wv, Wo))
    bq, bk, bv, bo = (np.asarray(a) for a in (bq, bk, bv, bo))

    if _CACHED_NC is None:
        _CACHED_NC = _build_nc()
    nc = _CACHED_NC

    # host-side shard prep (transposes + bf16 casts)
    xT_b = [np.ascontiguousarray(x[b].T).astype(NP_BF16) for b in range(B)]
    wqkT_g = [
        np.ascontiguousarray(
            np.concatenate(
                [Wq[g * FL : (g + 1) * FL, :].T, Wk[g * FL : (g + 1) * FL, :].T],
                axis=1,
            )
        ).astype(NP_BF16)
        for g in range(HG)
    ]
    wvT_g = [np.ascontiguousarray(Wv[g * FL : (g + 1) * FL, :].T).astype(NP_BF16) for g in range(HG)]
    woT_g = [np.ascontiguousarray(Wo[:, g * FL : (g + 1) * FL].T).astype(NP_BF16) for g in range(HG)]
    bq_g = [np.ascontiguousarray(bq[g * FL : (g + 1) * FL]).astype(np.float32) for g in range(HG)]
    bk_g = [np.ascontiguousarray(bk[g * FL : (g + 1) * FL]).astype(np.float32) for g in range(HG)]
    bv_g = [np.ascontiguousarray(bv[g * FL : (g + 1) * FL]).astype(np.float32) for g in range(HG)]
    bo_half = (bo.astype(np.float32) / 2.0)

    in_maps = []
    for c in range(NCORES):
        b, g = c // HG, c % HG
        in_maps.append(
            {
                "xT": xT_b[b],
                "wqkT": wqkT_g[g],
                "wvT": wvT_g[g],
                "woT": woT_g[g],
                "bq": bq_g[g],
                "bk": bk_g[g],
                "bv": bv_g[g],
                "boh": bo_half,
            }
        )

    res = run_bass_kernel_spmd(nc, in_maps, core_ids=list(range(NCORES)), trace=trace)
    LAST_EXEC_TIME_NS = res.exec_time_ns
    LAST_RES = res

    out = np.empty((B, N, F), np.float32)
    for b in range(B):
        out[b] = res.results[2 * b]["out"] + res.results[2 * b + 1]["out"]
    return out


# revision 24
# speedup vs baseline: 1.0348x; 1.0029x over previous
"""Multi-head self-attention (B=4, N=2048, F=1024, 16 heads) on 8 TRN2 NeuronCores.

Sharding (Megatron-style, per the hint): data-parallel over the 4 batches x
tensor-parallel split of the 16 heads into 2 groups of 8. Core c handles
batch c//2 with head group c%2 (512 of the 1024 qkv features, column-split
Wq/Wk/Wv, row-split Wo). Each core emits a partial o-projection [2048, 1024];
the host unshard sums the pair of partials per batch (the Megatron
all-reduce) and stacks batches.

Device kernel layout choices (all matmuls bf16 with f32 PSUM accumulation):
  xT   [1024f, 2048i]  (x transposed on host)  - rhs for q/k, lhsT for v
  qT/kT [512o, 2048i]  (o = head-major features, on partitions)
  vAug [2048j, 8h, 65] (per head: V columns 0..63 plus a ones column at 64
                        so the attention-value matmul also yields the softmax
                        denominator Z as output row 64)
  scores S^T [j, i] via lhsT=kT-chunk, rhs=qT; exp on ScalarE (scale=1/32,
  no max subtraction needed: |S/32| <~ 1.5); attnU^T accumulated over j in
  PSUM, copied to SBUF promptly to release the PSUM bank; softmax
  normalization (1/Z broadcast) runs off the critical path via small DMA
  round-trips through DRAM.

QKV is interleaved with attention per 128-row chunk of q/k so the ScalarE
exp stream (the steady-state bottleneck) starts early and the remaining
projections hide under it.
"""

import sys
import types

sys.path.insert(0, "/opt/trn_rl_repo")

import numpy as np

# Best-effort: register the axon NTFF profile hook so trace=True works
# (used by test harnesses via BASS_TRACE); harmless when tracing is off.
try:
    import antenv

    if "antenv.axon_hooks" not in sys.modules:
        from trn_agent_boot.trn_boot import _ntff_profile_via_ctypes

        _hooks_mod = types.ModuleType("antenv.axon_hooks")
        _hook = _ntff_profile_via_ctypes("/opt/axon/libaxon_pjrt.so")
        _hooks_mod.get_axon_ntff_profile_hook = lambda: _hook
        _hooks_mod.set_axon_ntff_profile_hook = lambda h: None
        sys.modules["antenv.axon_hooks"] = _hooks_mod
        antenv.axon_hooks = _hooks_mod
except Exception:
    pass

import concourse.bacc as bacc
import concourse.tile as tile
from concourse import mybir
from concourse.bass_utils import run_bass_kernel_spmd

B, N, F = 4, 2048, 1024
HEAD, HD = 16, 64
NCORES = 8
HG = 2                # head groups (tensor-parallel degree per batch)
FL = F // HG          # local features per core = 512
HL = HEAD // HG       # local heads per core = 8
OC = FL // 128        # o-chunks of 128 in qT/kT = 4
FT = F // 128         # f (contraction) tiles = 8
IC = N // 128         # i/j chunks of 128 = 16
ISPAN = 1024          # attention i-span per inner block
NSP = N // ISPAN      # spans = 2

BF16 = mybir.dt.bfloat16
F32 = mybir.dt.float32
NP_BF16 = mybir.dt.np(BF16)

_CACHED_NC = None
LAST_EXEC_TIME_NS = None
LAST_RES = None


def _build_nc():
    nc = bacc.Bacc("TRN2")

    xT = nc.dram_tensor("xT", [F, N], BF16, kind="ExternalInput")
    wqkT = nc.dram_tensor("wqkT", [F, 2 * FL], BF16, kind="ExternalInput")
    wvT = nc.dram_tensor("wvT", [F, FL], BF16, kind="ExternalInput")
    woT = nc.dram_tensor("woT", [FL, F], BF16, kind="ExternalInput")
    bq = nc.dram_tensor("bq", [FL], F32, kind="ExternalInput")
    bk = nc.dram_tensor("bk", [FL], F32, kind="ExternalInput")
    bv = nc.dram_tensor("bv", [FL], F32, kind="ExternalInput")
    boh = nc.dram_tensor("boh", [F], F32, kind="ExternalInput")
    out = nc.dram_tensor("out", [N, F], F32, kind="ExternalOutput")

    with tile.TileContext(nc) as tc:
        with (
            tc.tile_pool(name="big", bufs=1) as big,
            tc.tile_pool(name="etile", bufs=6) as epool,
            tc.tile_pool(name="auc", bufs=3) as aucp,
            tc.tile_pool(name="ztile", bufs=3) as zpool,
            tc.tile_pool(name="rbc", bufs=3) as rpool,
            tc.tile_pool(name="ostage", bufs=3) as opool,
            tc.tile_pool(name="dspill", bufs=4, space="DRAM") as dpool,
            tc.tile_pool(name="pp", bufs=2, space="PSUM") as pp,
            tc.tile_pool(name="sp", bufs=2, space="PSUM") as spp,
            tc.tile_pool(name="aup", bufs=1, space="PSUM") as aup,
        ):
            # ---- resident SBUF tensors ----
            xT_t = [big.tile([128, N], BF16, tag=f"xT{t}", name=f"xT{t}") for t in range(FT)]
            wqk_sb = big.tile([128, FT, 2 * FL], BF16, tag="wqk")
            wvT_sb = big.tile([128, FT, FL], BF16, tag="wvT")
            woT_sb = big.tile([128, OC, F], BF16, tag="woT")
            qT_oc = [big.tile([128, N], BF16, tag=f"qT{oc}", name=f"qT{oc}") for oc in range(OC)]
            kT_oc = [big.tile([128, N], BF16, tag=f"kT{oc}", name=f"kT{oc}") for oc in range(OC)]
            vAug_ic = [
                big.tile([128, HL, HD + 1], BF16, tag=f"vAug{ic}", name=f"vAug{ic}")
                for ic in range(IC)
            ]
            # pairs 0..2 write full attnT rows; pair 3 is chased by the output
            # projection span by span, so its attnT is split per 512-i span
            attnT_oc = [big.tile([128, N], BF16, tag=f"attnT{oc}", name=f"attnT{oc}") for oc in range(3)]
            attnT3_s = [
                big.tile([128, 512], BF16, tag=f"attnT3s{s}", name=f"attnT3s{s}")
                for s in range(N // 512)
            ]
            bq_sb = big.tile([128, OC], F32, tag="bq")
            bk_sb = big.tile([128, OC], F32, tag="bk")
            bv_sb = big.tile([128, FL], F32, tag="bv")
            bo_sb = big.tile([128, F], F32, tag="bo")

            # ---- input DMAs (small ones first; xT split per f-tile so
            # compute starts as soon as its operands land) ----
            wqkr = wqkT.rearrange("(t p) o -> p t o", p=128)
            nc.sync.dma_start(out=wqk_sb[:, :, FL:], in_=wqkr[:, :, FL:])
            nc.sync.dma_start(out=wqk_sb[:, :, 0:FL], in_=wqkr[:, :, 0:FL])
            xTr = xT.rearrange("(t p) i -> p t i", p=128)
            for t in range(FT):
                nc.sync.dma_start(out=xT_t[t][:], in_=xTr[:, t, :])
            nc.sync.dma_start(
                out=wvT_sb[:], in_=wvT.rearrange("(t p) o -> p t o", p=128)
            )
            nc.sync.dma_start(
                out=bq_sb[:], in_=bq.rearrange("(c p) -> p c", p=128)
            )
            nc.sync.dma_start(
                out=bk_sb[:], in_=bk.rearrange("(c p) -> p c", p=128)
            )
            nc.sync.dma_start(out=bv_sb[:], in_=bv[None, :].partition_broadcast(128))
            nc.sync.dma_start(out=bo_sb[:], in_=boh[None, :].partition_broadcast(128))
            nc.sync.dma_start(
                out=woT_sb[:], in_=woT.rearrange("(t p) g -> p t g", p=128)
            )
            # ones column (64) for every head; V overwrites columns 0..63
            for ic in range(IC):
                nc.vector.memset(vAug_ic[ic][:], 1.0)

            # PE warmup: dummy matmuls while the input DMAs are in flight so
            # the HAM clock-gate reaches 2.4 GHz before the real work starts
            # (otherwise the first ~3.4us of projections run at half clock).
            wup = big.tile([128, 128], BF16, tag="wup")
            nc.vector.memset(wup[:], 0.0)
            wup5 = big.tile([128, 512], BF16, tag="wup5")
            nc.vector.memset(wup5[:], 0.0)
            wps = pp.tile([128, 512], F32, tag="pp", name="wps")
            for w in range(60):
                nc.tensor.matmul(
                    wps[:],
                    lhsT=wup[:],
                    rhs=wup5[:],
                    start=True,
                    stop=True,
                )

            def project_v_chunk(ic):
                ps = pp.tile([128, 512], F32, tag="pp")
                for t in range(FT):
                    nc.tensor.matmul(
                        ps[:],
                        lhsT=xT_t[t][:, ic * 128 : (ic + 1) * 128],
                        rhs=wvT_sb[:, t, :],
                        start=(t == 0),
                        stop=(t == FT - 1),
                    )
                nc.vector.tensor_add(
                    out=vAug_ic[ic][:, :, 0:HD],
                    in0=ps.rearrange("p (h d) -> p h d", h=HL),
                    in1=bv_sb.rearrange("p (h d) -> p h d", h=HL),
                )

            def project_qk(base, b_sb, dst, oc):
                for ic in range(N // 512):
                    ps = pp.tile([128, 512], F32, tag="pp")
                    for t in range(FT):
                        nc.tensor.matmul(
                            ps[:],
                            lhsT=wqk_sb[:, t, base + oc * 128 : base + (oc + 1) * 128],
                            rhs=xT_t[t][:, ic * 512 : (ic + 1) * 512],
                            start=(t == 0),
                            stop=(t == FT - 1),
                        )
                    nc.vector.tensor_scalar_add(
                        out=dst[:, ic * 512 : (ic + 1) * 512],
                        in0=ps[:],
                        scalar1=b_sb[:, oc : oc + 1],
                    )

            def attention_pair(oc, chase=None, per_j0=None):
                # heads h0 = 2*oc (q/k rows 0:64) and h1 = 2*oc+1 (rows 64:128)
                # are processed together: their score matmuls sit on disjoint
                # PE row-groups (K=64 at base partition 0 vs 64) and run
                # concurrently; one [128, 1024] S-PSUM tile holds a 512-wide
                # i-span for each head so exp still works in [128,1024] calls.
                # Score matmuls are emitted one step ahead of the attn-value
                # matmuls so the PE never parks behind an exp-blocked AV and
                # the exp stream stays gapless across span boundaries.
                h0, h1 = 2 * oc, 2 * oc + 1

                def s_emit(isp, j, st):
                    i0 = isp * 512
                    nc.tensor.matmul(
                        st[:, 0:512],
                        lhsT=kT_oc[oc][0:64, j * 128 : (j + 1) * 128],
                        rhs=qT_oc[oc][0:64, i0 : i0 + 512],
                        start=True,
                        stop=True,
                    )
                    nc.tensor.matmul(
                        st[:, 512:1024],
                        lhsT=kT_oc[oc][64:128, j * 128 : (j + 1) * 128],
                        rhs=qT_oc[oc][64:128, i0 : i0 + 512],
                        start=True,
                        stop=True,
                    )

                nxt = spp.tile([128, 1024], F32, tag="sp", name="st")
                s_emit(0, 0, nxt)
                for isp in range(N // 512):
                    i0 = isp * 512
                    au = aup.tile([HD + 1, 1024], F32, tag="au")
                    for j in range(IC):
                        st = nxt
                        eT = epool.tile([128, 1024], BF16, tag="eT")
                        nc.scalar.activation(
                            eT[:], st[:], mybir.ActivationFunctionType.Exp,
                            scale=1.0 / 32.0,
                        )
                        if j + 1 < IC:
                            nxt = spp.tile([128, 1024], F32, tag="sp", name="st")
                            s_emit(isp, j + 1, nxt)
                        elif isp + 1 < N // 512:
                            nxt = spp.tile([128, 1024], F32, tag="sp", name="st")
                            s_emit(isp + 1, 0, nxt)
                        if isp == 0 and per_j0 is not None:
                            per_j0(j)
                        nc.tensor.matmul(
                            au[:, 0:512],
                            lhsT=vAug_ic[j][:, h0, :],
                            rhs=eT[:, 0:512],
                            start=(j == 0),
                            stop=(j == IC - 1),
                        )
                        nc.tensor.matmul(
                            au[:, 512:1024],
                            lhsT=vAug_ic[j][:, h1, :],
                            rhs=eT[:, 512:1024],
                            start=(j == 0),
                            stop=(j == IC - 1),
                        )
                    # copy attnU + Z out of PSUM promptly to release the bank
                    auc = aucp.tile([HD + 1, 1024], F32, tag="auc")
                    nc.vector.tensor_copy(auc[:], au[:])
                    # 1/Z with decent parallelism: bounce Z through DRAM into
                    # a [128, 8] layout, reciprocal, bounce back broadcast
                    zd = dpool.tile([1, 1024], F32, tag="zd")
                    nc.sync.dma_start(out=zd[:], in_=auc[HD : HD + 1, :])
                    zs = zpool.tile([128, 8], F32, tag="zs")
                    nc.sync.dma_start(
                        out=zs[:], in_=zd[0, :].rearrange("(p f) -> p f", p=128)
                    )
                    zr = zpool.tile([128, 8], F32, tag="zr")
                    nc.vector.reciprocal(zr[:], zs[:])
                    zrd = dpool.tile([1, 1024], F32, tag="zrd")
                    nc.sync.dma_start(
                        out=zrd[0, :].rearrange("(p f) -> p f", p=128), in_=zr[:]
                    )
                    rb = rpool.tile([64, 1024], F32, tag="rb")
                    nc.sync.dma_start(
                        out=rb[:], in_=zrd[0, :].partition_broadcast(64)
                    )
                    if oc < 3:
                        dst0 = attnT_oc[oc][0:64, i0 : i0 + 512]
                        dst1 = attnT_oc[oc][64:128, i0 : i0 + 512]
                    else:
                        dst0 = attnT3_s[isp][0:64, :]
                        dst1 = attnT3_s[isp][64:128, :]
                    nc.vector.tensor_mul(out=dst0, in0=auc[0:HD, 0:512], in1=rb[:, 0:512])
                    nc.vector.tensor_mul(out=dst1, in0=auc[0:HD, 512:1024], in1=rb[:, 512:1024])
                    if chase is not None:
                        chase(isp)

            def oproj_span(isp):
                # output projection for i in [isp*512, (isp+1)*512)
                for lic in range(4):
                    ic = isp * 4 + lic
                    for gc in range(F // 512):
                        ps = pp.tile([128, 512], F32, tag="pp")
                        for ct in range(OC):
                            if ct < 3:
                                lhsT = attnT_oc[ct][:, ic * 128 : (ic + 1) * 128]
                            else:
                                lhsT = attnT3_s[isp][:, lic * 128 : (lic + 1) * 128]
                            nc.tensor.matmul(
                                ps[:],
                                lhsT=lhsT,
                                rhs=woT_sb[:, ct, gc * 512 : (gc + 1) * 512],
                                start=(ct == 0),
                                stop=(ct == OC - 1),
                            )
                        st = opool.tile([128, 512], F32, tag="ost")
                        nc.vector.tensor_add(
                            out=st[:], in0=ps[:], in1=bo_sb[:, gc * 512 : (gc + 1) * 512]
                        )
                        nc.sync.dma_start(
                            out=out[ic * 128 : (ic + 1) * 128, gc * 512 : (gc + 1) * 512],
                            in_=st[:],
                        )

            # ---- interleaved projections + attention; o-proj chases pair 3.
            # The v projection is produced just-in-time inside pair 0's first
            # span (attn-value matmul at chunk j only needs v chunk j), so the
            # exp stream starts right after q0/k0 instead of after all of v.
            project_qk(FL, bk_sb, kT_oc[0], 0)
            project_qk(0, bq_sb, qT_oc[0], 0)
            project_v_chunk(0)

            def v_jit(j):
                if j + 1 < IC:
                    project_v_chunk(j + 1)

            attention_pair(0, per_j0=v_jit)
            for oc in range(1, 3):
                project_qk(0, bq_sb, qT_oc[oc], oc)
                project_qk(FL, bk_sb, kT_oc[oc], oc)
                attention_pair(oc)
            project_qk(0, bq_sb, qT_oc[3], 3)
            project_qk(FL, bk_sb, kT_oc[3], 3)
            attention_pair(
                3, chase=lambda isp: oproj_span(isp - 1) if isp >= 1 else None
            )
            oproj_span(N // 512 - 1)

    nc.finalize()
    return nc


def kernel(x, Wq, bq, Wk, bk, Wv, bv, Wo, bo, trace=False):
    global _CACHED_NC, LAST_EXEC_TIME_NS, LAST_RES
    x = np.asarray(x)
    Wq, Wk, Wv, Wo = (np.asarray(a) for a in (Wq, Wk, Wv, Wo))
    bq, bk, bv, bo = (np.asarray(a) for a in (bq, bk, bv, bo))

    if _CACHED_NC is None:
        _CACHED_NC = _build_nc()
    nc = _CACHED_NC

    # host-side shard prep (transposes + bf16 casts)
    xT_b = [np.ascontiguousarray(x[b].T).astype(NP_BF16) for b in range(B)]
    wqkT_g = [
        np.ascontiguousarray(
            np.concatenate(
                [Wq[g * FL : (g + 1) * FL, :].T, Wk[g * FL : (g + 1) * FL, :].T],
                axis=1,
            )
        ).astype(NP_BF16)
        for g in range(HG)
    ]
    wvT_g = [np.ascontiguousarray(Wv[g * FL : (g + 1) * FL, :].T).astype(NP_BF16) for g in range(HG)]
    woT_g = [np.ascontiguousarray(Wo[:, g * FL : (g + 1) * FL].T).astype(NP_BF16) for g in range(HG)]
    bq_g = [np.ascontiguousarray(bq[g * FL : (g + 1) * FL]).astype(np.float32) for g in range(HG)]
    bk_g = [np.ascontiguousarray(bk[g * FL : (g + 1) * FL]).astype(np.float32) for g in range(HG)]
    bv_g = [np.ascontiguousarray(bv[g * FL : (g + 1) * FL]).astype(np.float32) for g in range(HG)]
    bo_half = (bo.astype(np.float32) / 2.0)

    in_maps = []
    for c in range(NCORES):
        b, g = c // HG, c % HG
        in_maps.append(
            {
                "xT": xT_b[b],
                "wqkT": wqkT_g[g],
                "wvT": wvT_g[g],
                "woT": woT_g[g],
                "bq": bq_g[g],
                "bk": bk_g[g],
                "bv": bv_g[g],
                "boh": bo_half,
            }
        )

    res = run_bass_kernel_spmd(nc, in_maps, core_ids=list(range(NCORES)), trace=trace)
    LAST_EXEC_TIME_NS = res.exec_time_ns
    LAST_RES = res

    out = np.empty((B, N, F), np.float32)
    for b in range(B):
        out[b] = res.results[2 * b]["out"] + res.results[2 * b + 1]["out"]
    return out
